# revision 4
# baseline (speedup 1.0000x reference)
"""CuttleNet Trainium2 kernel.

Strategy:
- Data-parallel over the 262144-row batch across 8 NeuronCores (32768 rows each).
- Host folds the gene-column gathers into expanded weight matrices (scatter-add
  over the index tables), so the device only does dense matmuls on x.
- Everything on device is computed feature-major ([features, batch] with batch on
  the free dimension, 512-column tiles = one PSUM bank). The host pre-transposes
  x to xT [300, B] (bf16) and post-transposes the [130, B] fp32 result back.
- The 8 per-class MLPs are packed into 3 partition groups ({c0}=80, {c1..c4}=120,
  {c5,c6,c7}=60 rows of 2S), hidden layers become block-diagonal matmuls, and
  log-softmax is done with ones-matmul partition reductions plus selector-matmul
  broadcasts of the per-class log-sums / class log-probs.
"""

import numpy as np
import ml_dtypes

F16 = np.float16

NUM_CLASSES = 8
SUB_S = [40, 30, 15, 10, 5, 10, 19, 1]          # S_c, sums to 130
T2 = [2 * s for s in SUB_S]                      # 2*S_c, sums to 260
GROUPS = [[0], [1, 2, 3, 4], [5, 6, 7]]
GSIZE = [sum(T2[c] for c in g) for g in GROUPS]  # [80, 120, 60]
GBASE = [0, 80, 200]                             # col offset of group in the 260
OUT_OFF = np.concatenate([[0], np.cumsum(SUB_S)])  # [0,40,70,85,95,100,110,129,130]
N_CORES = 8
B_TOTAL = 262144
B_CORE = B_TOTAL // N_CORES                      # 32768
BT = 512                                         # batch tile (one PSUM bank fp32)
NT = B_CORE // BT                                # 64 tiles per core


def _f32(a):
    return np.asarray(a, dtype=np.float32)


def _block_diag(mats):
    n = sum(m.shape[0] for m in mats)
    k = sum(m.shape[1] for m in mats)
    out = np.zeros((n, k), np.float32)
    r = c = 0
    for m in mats:
        out[r:r + m.shape[0], c:c + m.shape[1]] = m
        r += m.shape[0]
        c += m.shape[1]
    return out


def prep_weights(class_gene_idx, sub_gene_idx, params):
    """Expand/repack all model weights into the device layout (fp32)."""
    cg = np.asarray(class_gene_idx)
    sg = np.asarray(sub_gene_idx)
    cW1, cb1 = (_f32(a) for a in params["class_fc1"])   # [60,16], [16]
    cW2, cb2 = (_f32(a) for a in params["class_fc2"])   # [16,8], [8]
    subnets = params["subnets"]

    # fc1, expanded over gene gathers. cols 0:260 subnets (group order), 260:276 class head
    W1exp = np.zeros((300, 276), np.float32)
    np.add.at(W1exp[:, 260:276], cg, cW1)
    wco = np.zeros((8, 260), np.float32)   # class_out -> subnet fc1 contribution
    b1 = np.zeros(260, np.float32)
    for g, classes in enumerate(GROUPS):
        off = GBASE[g]
        for c in classes:
            W, b = subnets[c]["fc1"]
            W = _f32(W)                     # [108, 2S]
            np.add.at(W1exp[:, off:off + T2[c]], sg[c], W[:100])
            wco[:, off:off + T2[c]] = W[100:108]
            b1[off:off + T2[c]] = _f32(b)
            off += T2[c]

    # hidden layers: block-diagonal per group
    H1 = [_block_diag([_f32(subnets[c]["hidden"][0][0]) for c in g]) for g in GROUPS]
    H2 = [_block_diag([_f32(subnets[c]["hidden"][1][0]) for c in g]) for g in GROUPS]
    bh1 = [np.concatenate([_f32(subnets[c]["hidden"][0][1]) for c in g]) for g in GROUPS]
    bh2 = [np.concatenate([_f32(subnets[c]["hidden"][1][1]) for c in g]) for g in GROUPS]

    # fc2: scatter group rows -> global output columns (0:128 in M0, 128:130 in M1)
    W2M0 = [np.zeros((GSIZE[g], 128), np.float32) for g in range(3)]
    W2M1 = np.zeros((GSIZE[2], 2), np.float32)
    b2 = np.zeros(130, np.float32)
    for g, classes in enumerate(GROUPS):
        lr = 0
        for c in classes:
            W, b = subnets[c]["fc2"]
            W = _f32(W)                     # [2S, S]
            for j in range(SUB_S[c]):
                col = OUT_OFF[c] + j
                if col < 128:
                    W2M0[g][lr:lr + T2[c], col] = W[:, j]
                else:
                    W2M1[lr:lr + T2[c], col - 128] = W[:, j]
            b2[OUT_OFF[c]:OUT_OFF[c] + SUB_S[c]] = _f32(b)
            lr += T2[c]

    # softmax selectors
    cls_of_row = np.zeros(130, np.int64)
    for c in range(NUM_CLASSES):
        cls_of_row[OUT_OFF[c]:OUT_OFF[c + 1]] = c
    SB = np.zeros((130, 8), np.float32)
    SB[np.arange(130), cls_of_row] = 1.0
    BL = SB.T.copy()                        # [8, 130]

    return {
        "W1exp": W1exp, "wco": wco, "b1": b1, "bc1": cb1, "wc2": cW2, "bc2": cb2,
        "H1": H1, "H2": H2, "bh1": bh1, "bh2": bh2,
        "W2M0": W2M0, "W2M1": W2M1, "b2": b2,
        "sb0": SB[:128], "sb1": SB[128:], "bl0": BL[:, :128], "bl1": BL[:, 128:],
        "ones8": np.ones((8, 1), np.float32), "ones1x8": np.ones((1, 8), np.float32),
    }


def numpy_forward(w, x):
    """Reference implementation of the device math (fp32 numpy), for validation."""
    relu = lambda v: np.maximum(v, 0.0)
    zc = x @ w["W1exp"][:, 260:276] + w["bc1"]
    ch = relu(zc)
    z2 = ch @ w["wc2"] + w["bc2"]
    ls = np.log(np.exp(z2).sum(1, keepdims=True))
    co = z2 - ls                                    # class log-probs [n, 8]
    h1 = relu(x @ w["W1exp"][:, :260] + co @ w["wco"] + w["b1"])
    H1 = _block_diag(w["H1"]); H2 = _block_diag(w["H2"])
    h2 = relu(h1 @ H1 + np.concatenate(w["bh1"]))
    h3 = relu(h2 @ H2 + np.concatenate(w["bh2"]))
    h4 = h3 + h1
    z = np.zeros((x.shape[0], 130), np.float32)
    for g in range(3):
        hg = h4[:, GBASE[g]:GBASE[g] + GSIZE[g]]
        z[:, :128] += hg @ w["W2M0"][g]
    z[:, 128:] += h4[:, GBASE[2]:GBASE[2] + GSIZE[2]] @ w["W2M1"]
    z += w["b2"]
    es = np.exp(z)
    SB = np.concatenate([w["sb0"], w["sb1"]], 0)    # [130, 8]
    s = es @ SB                                     # per-class sums [n, 8]
    lss = np.log(s)
    LS = lss @ np.concatenate([w["bl0"], w["bl1"]], 1)   # [n, 130]
    CO = co @ np.concatenate([w["bl0"], w["bl1"]], 1)    # [n, 130]
    return (z - LS) * CO


# ---------------------------------------------------------------------------
# device program
# ---------------------------------------------------------------------------

def _dev_weight_arrays(w):
    """Cast to device dtypes. Matmul lhsT operands -> bf16, biases -> fp32 [P,1]."""
    bf = lambda a: np.ascontiguousarray(np.asarray(a, np.float32), dtype=np.float32).astype(F16)
    col = lambda a: np.ascontiguousarray(np.asarray(a, np.float32).reshape(-1, 1))
    d = {
        "w1_k0": bf(w["W1exp"][0:128]),
        "w1_k1": bf(w["W1exp"][128:256]),
        "w1_k2": bf(w["W1exp"][256:300]),
        "wco": bf(w["wco"]),
        "wc2": bf(w["wc2"]),
        "h1g0": bf(w["H1"][0]), "h1g1": bf(w["H1"][1]), "h1g2": bf(w["H1"][2]),
        "h2g0": bf(w["H2"][0]), "h2g1": bf(w["H2"][1]), "h2g2": bf(w["H2"][2]),
        "w2m0g0": bf(w["W2M0"][0]), "w2m0g1": bf(w["W2M0"][1]), "w2m0g2": bf(w["W2M0"][2]),
        "w2m1": bf(w["W2M1"]),
        "sb0": bf(w["sb0"]), "sb1": bf(w["sb1"]),
        "bl0": bf(w["bl0"]), "bl1": bf(w["bl1"]),
        "ones8": bf(w["ones8"]), "ones1x8": bf(w["ones1x8"]),
        "bc1_b": col(w["bc1"]), "bc2_b": col(w["bc2"]),
        "b1g0_b": col(w["b1"][0:80]), "b1g1_b": col(w["b1"][80:200]), "b1g2_b": col(w["b1"][200:260]),
        "bh1g0_b": col(w["bh1"][0]), "bh1g1_b": col(w["bh1"][1]), "bh1g2_b": col(w["bh1"][2]),
        "bh2g0_b": col(w["bh2"][0]), "bh2g1_b": col(w["bh2"][1]), "bh2g2_b": col(w["bh2"][2]),
        "b2m0_b": col(w["b2"][0:128]), "b2m1_b": col(w["b2"][128:130]),
    }
    return d


def build_nc(n_tiles=NT, bt=BT):
    import concourse.bacc as bacc
    import concourse.tile as tile
    import concourse.mybir as mybir

    dt = mybir.dt
    AF = mybir.ActivationFunctionType
    ALU = mybir.AluOpType
    B = n_tiles * bt

    nc = bacc.Bacc("TRN2", target_bir_lowering=False, debug=False,
                   num_devices=N_CORES)

    xT = nc.dram_tensor("xT", [300, B], dt.float16, kind="ExternalInput")
    outT = nc.dram_tensor("outT", [130, B], dt.float32, kind="ExternalOutput")

    wspec = {
        "w1_k0": ([128, 276], dt.float16), "w1_k1": ([128, 276], dt.float16),
        "w1_k2": ([44, 276], dt.float16),
        "wco": ([8, 260], dt.float16), "wc2": ([16, 8], dt.float16),
        "h1g0": ([80, 80], dt.float16), "h1g1": ([120, 120], dt.float16), "h1g2": ([60, 60], dt.float16),
        "h2g0": ([80, 80], dt.float16), "h2g1": ([120, 120], dt.float16), "h2g2": ([60, 60], dt.float16),
        "w2m0g0": ([80, 128], dt.float16), "w2m0g1": ([120, 128], dt.float16), "w2m0g2": ([60, 128], dt.float16),
        "w2m1": ([60, 2], dt.float16),
        "sb0": ([128, 8], dt.float16), "sb1": ([2, 8], dt.float16),
        "bl0": ([8, 128], dt.float16), "bl1": ([8, 2], dt.float16),
        "ones8": ([8, 1], dt.float16), "ones1x8": ([1, 8], dt.float16),
        "bc1_b": ([16, 1], dt.float32), "bc2_b": ([8, 1], dt.float32),
        "b1g0_b": ([80, 1], dt.float32), "b1g1_b": ([120, 1], dt.float32), "b1g2_b": ([60, 1], dt.float32),
        "bh1g0_b": ([80, 1], dt.float32), "bh1g1_b": ([120, 1], dt.float32), "bh1g2_b": ([60, 1], dt.float32),
        "bh2g0_b": ([80, 1], dt.float32), "bh2g1_b": ([120, 1], dt.float32), "bh2g2_b": ([60, 1], dt.float32),
        "b2m0_b": ([128, 1], dt.float32), "b2m1_b": ([2, 1], dt.float32),
    }
    wdram = {k: nc.dram_tensor(k, shape, d, kind="ExternalInput")
             for k, (shape, d) in wspec.items()}

    GG = GSIZE                      # [80, 120, 60]
    mm = nc.tensor.matmul

    with tile.TileContext(nc) as tc:
        with tc.tile_pool(name="wp", bufs=1) as wp, \
             tc.tile_pool(name="io", bufs=3) as io, \
             tc.tile_pool(name="act", bufs=2) as acp, \
             tc.tile_pool(name="outp", bufs=3) as outp, \
             tc.tile_pool(name="ps", bufs=5, space="PSUM") as ps, \
             tc.tile_pool(name="pst", bufs=1, space="PSUM") as pst:

            wt = {}
            for k, (shape, d) in wspec.items():
                t = wp.tile(shape, d, tag=k)
                nc.sync.dma_start(out=t[:], in_=wdram[k].ap())
                wt[k] = t

            for it in range(n_tiles):
                c0, c1 = it * bt, (it + 1) * bt

                # --- load xT chunks ---
                x0 = io.tile([128, bt], dt.float16, tag="x0")
                x1 = io.tile([128, bt], dt.float16, tag="x1")
                x2 = io.tile([44, bt], dt.float16, tag="x2")
                nc.sync.dma_start(out=x0[:], in_=xT.ap()[0:128, c0:c1])
                nc.sync.dma_start(out=x1[:], in_=xT.ap()[128:256, c0:c1])
                nc.sync.dma_start(out=x2[:], in_=xT.ap()[256:300, c0:c1])
                xchunks = [x0, x1, x2]

                # --- class head ---
                psA = ps.tile([16, bt], dt.float32, tag="big")
                for k, xc in enumerate(xchunks):
                    mm(psA[:], wt["w1_k%d" % k][:, 260:276], xc[:],
                       start=(k == 0), stop=(k == 2))
                ch = acp.tile([16, bt], dt.float16, tag="ch")
                nc.scalar.activation(ch[:], psA[:], AF.Relu, bias=wt["bc1_b"][:])

                tailA = pst.tile([128, bt], dt.float32, tag="tailA")
                psB = tailA[0:8]         # class fc2 logits
                psC = tailA[32:33]       # sum(exp)
                psD = tailA[64:72]       # broadcast(ls_cls)
                mm(psB, wt["wc2"][:], ch[:], start=True, stop=True)
                e_cls = acp.tile([8, bt], dt.float16, tag="e_cls")
                nc.scalar.activation(e_cls[:], psB, AF.Exp, bias=wt["bc2_b"][:])
                z_cls = acp.tile([8, bt], dt.float32, tag="z_cls")
                nc.vector.tensor_scalar_add(z_cls[:], psB, wt["bc2_b"][:])
                mm(psC, wt["ones8"][:], e_cls[:], start=True, stop=True)
                ls_cls = acp.tile([1, bt], dt.float16, tag="ls_cls")
                nc.scalar.activation(ls_cls[:], psC, AF.Ln)
                mm(psD, wt["ones1x8"][:], ls_cls[:], start=True, stop=True)
                co = acp.tile([8, bt], dt.float16, tag="co")
                nc.vector.tensor_sub(co[:], z_cls[:], psD)

                # --- subnet fc1 (3 groups, 3 x-chunks + class_out chunk) ---
                psG = []
                for g in range(3):
                    pg = ps.tile([GG[g], bt], dt.float32, tag="big")
                    lo, hi = GBASE[g], GBASE[g] + GG[g]
                    for k, xc in enumerate(xchunks):
                        mm(pg[:], wt["w1_k%d" % k][:, lo:hi], xc[:],
                           start=(k == 0), stop=False)
                    mm(pg[:], wt["wco"][:, lo:hi], co[:], start=False, stop=True)
                    psG.append(pg)
                h1 = []
                for g in range(3):
                    t = acp.tile([GG[g], bt], dt.float16, tag="h1g%d" % g)
                    nc.scalar.activation(t[:], psG[g][:], AF.Relu, bias=wt["b1g%d_b" % g][:])
                    h1.append(t)

                # --- hidden 1 (ACT relu) ---
                h2 = []
                for g in range(3):
                    pg = ps.tile([GG[g], bt], dt.float32, tag="big")
                    mm(pg[:], wt["h1g%d" % g][:], h1[g][:], start=True, stop=True)
                    t = acp.tile([GG[g], bt], dt.float16, tag="h2g%d" % g)
                    nc.scalar.activation(t[:], pg[:], AF.Relu, bias=wt["bh1g%d_b" % g][:])
                    h2.append(t)

                # --- hidden 2 (DVE relu) + skip ---
                h4 = []
                for g in range(3):
                    pg = ps.tile([GG[g], bt], dt.float32, tag="big")
                    mm(pg[:], wt["h2g%d" % g][:], h2[g][:], start=True, stop=True)
                    t3 = acp.tile([GG[g], bt], dt.float16, tag="h3g%d" % g)
                    nc.vector.tensor_scalar(t3[:], pg[:], wt["bh2g%d_b" % g][:], 0.0,
                                            ALU.add, ALU.max)
                    t4 = acp.tile([GG[g], bt], dt.float16, tag="h4g%d" % g)
                    nc.vector.tensor_add(t4[:], t3[:], h1[g][:])
                    h4.append(t4)

                # --- fc2 ---
                psZ0 = ps.tile([128, bt], dt.float32, tag="big")
                for g in range(3):
                    mm(psZ0[:], wt["w2m0g%d" % g][:], h4[g][:],
                       start=(g == 0), stop=(g == 2))
                tailB = pst.tile([128, bt], dt.float32, tag="tailB")
                psZ1 = tailB[0:2]
                psS = tailB[32:40]
                psL1 = tailB[64:66]
                mm(psZ1, wt["w2m1"][:], h4[2][:], start=True, stop=True)

                # --- subnet softmax ---
                e0 = acp.tile([128, bt], dt.float16, tag="e0")
                e1 = acp.tile([2, bt], dt.float16, tag="e1")
                nc.scalar.activation(e0[:], psZ0[:], AF.Exp, bias=wt["b2m0_b"][:])
                nc.scalar.activation(e1[:], psZ1, AF.Exp, bias=wt["b2m1_b"][:])
                z0 = acp.tile([128, bt], dt.float32, tag="z0")
                z1 = acp.tile([2, bt], dt.float32, tag="z1")
                nc.vector.tensor_scalar_add(z0[:], psZ0[:], wt["b2m0_b"][:])
                nc.vector.tensor_scalar_add(z1[:], psZ1, wt["b2m1_b"][:])
                mm(psS, wt["sb0"][:], e0[:], start=True, stop=False)
                mm(psS, wt["sb1"][:], e1[:], start=False, stop=True)
                ls_sub = acp.tile([8, bt], dt.float16, tag="ls_sub")
                nc.scalar.activation(ls_sub[:], psS, AF.Ln)

                psL0 = ps.tile([128, bt], dt.float32, tag="big")
                mm(psL0[:], wt["bl0"][:], ls_sub[:], start=True, stop=True)
                mm(psL1, wt["bl1"][:], ls_sub[:], start=True, stop=True)
                psO0 = ps.tile([128, bt], dt.float32, tag="big")
                tailC = pst.tile([2, bt], dt.float32, tag="tailC")
                mm(psO0[:], wt["bl0"][:], co[:], start=True, stop=True)
                mm(tailC[:], wt["bl1"][:], co[:], start=True, stop=True)

                d0 = acp.tile([128, bt], dt.float32, tag="d0")
                d1 = acp.tile([2, bt], dt.float32, tag="d1")
                nc.vector.tensor_sub(d0[:], z0[:], psL0[:])
                nc.vector.tensor_sub(d1[:], z1[:], psL1)
                o0 = outp.tile([128, bt], dt.float32, tag="o0")
                o1 = outp.tile([2, bt], dt.float32, tag="o1")
                nc.vector.tensor_mul(o0[:], d0[:], psO0[:])
                nc.vector.tensor_mul(o1[:], d1[:], tailC[:])

                nc.sync.dma_start(out=outT.ap()[0:128, c0:c1], in_=o0[:])
                nc.sync.dma_start(out=outT.ap()[128:130, c0:c1], in_=o1[:])

    nc.finalize()
    return nc


_EXEC_TIME_NS = None


def kernel(x, class_gene_idx, sub_gene_idx, params):
    global _EXEC_TIME_NS
    import os
    from concourse import bass_utils

    x = np.asarray(x)
    w = prep_weights(class_gene_idx, sub_gene_idx, params)
    dev_w = _dev_weight_arrays(w)

    xbf = np.asarray(x, np.float32).astype(F16)        # [262144, 300] bf16

    nc = build_nc(NT, BT)

    in_maps = []
    for c in range(N_CORES):
        slab = np.ascontiguousarray(xbf[c * B_CORE:(c + 1) * B_CORE].T)  # [300, 32768]
        in_maps.append({**dev_w, "xT": slab})

    trace = bool(int(os.environ.get("CUTTLE_TRACE", "0")))
    res = bass_utils.run_bass_kernel_spmd(nc, in_maps, core_ids=list(range(N_CORES)),
                                          trace=trace)
    _EXEC_TIME_NS = res.exec_time_ns

    out = np.empty((B_TOTAL, 130), np.float32)
    for c in range(N_CORES):
        out[c * B_CORE:(c + 1) * B_CORE] = res.results[c]["outT"].T
    return out


# revision 9
# speedup vs baseline: 1.5747x; 1.5747x over previous
"""CuttleNet Trainium2 kernel.

Strategy:
- Data-parallel over the 262144-row batch across 8 NeuronCores (32768 rows each).
- Host folds the gene-column gathers into expanded weight matrices (scatter-add
  over the index tables), so the device only does dense matmuls on x.
- Everything on device is computed feature-major ([features, batch] with batch on
  the free dimension, 512-column tiles = one PSUM bank). The host pre-transposes
  x to xT [300, B] (bf16) and post-transposes the [130, B] fp32 result back.
- The 8 per-class MLPs are packed into 3 partition groups ({c0}=80, {c1..c4}=120,
  {c5,c6,c7}=60 rows of 2S), hidden layers become block-diagonal matmuls, and
  log-softmax is done with ones-matmul partition reductions plus selector-matmul
  broadcasts of the per-class log-sums / class log-probs.
"""

import numpy as np
import ml_dtypes

F16 = np.float16

NUM_CLASSES = 8
SUB_S = [40, 30, 15, 10, 5, 10, 19, 1]          # S_c, sums to 130
T2 = [2 * s for s in SUB_S]                      # 2*S_c, sums to 260
GROUPS = [[0], [1, 2, 3, 4], [5, 6, 7]]
GSIZE = [sum(T2[c] for c in g) for g in GROUPS]  # [80, 120, 60]
GBASE = [0, 80, 200]                             # col offset of group in the 260
OUT_OFF = np.concatenate([[0], np.cumsum(SUB_S)])  # [0,40,70,85,95,100,110,129,130]
N_CORES = 8
B_TOTAL = 262144
B_CORE = B_TOTAL // N_CORES                      # 32768
BT = 512                                         # batch tile (one PSUM bank fp32)
NT = B_CORE // BT                                # 64 tiles per core


def _f32(a):
    return np.asarray(a, dtype=np.float32)


def _block_diag(mats):
    n = sum(m.shape[0] for m in mats)
    k = sum(m.shape[1] for m in mats)
    out = np.zeros((n, k), np.float32)
    r = c = 0
    for m in mats:
        out[r:r + m.shape[0], c:c + m.shape[1]] = m
        r += m.shape[0]
        c += m.shape[1]
    return out


def prep_weights(class_gene_idx, sub_gene_idx, params):
    """Expand/repack all model weights into the device layout (fp32)."""
    cg = np.asarray(class_gene_idx)
    sg = np.asarray(sub_gene_idx)
    cW1, cb1 = (_f32(a) for a in params["class_fc1"])   # [60,16], [16]
    cW2, cb2 = (_f32(a) for a in params["class_fc2"])   # [16,8], [8]
    subnets = params["subnets"]

    # fc1, expanded over gene gathers. cols 0:260 subnets (group order), 260:276 class head
    W1exp = np.zeros((300, 276), np.float32)
    np.add.at(W1exp[:, 260:276], cg, cW1)
    wco = np.zeros((8, 260), np.float32)   # class_out -> subnet fc1 contribution
    b1 = np.zeros(260, np.float32)
    for g, classes in enumerate(GROUPS):
        off = GBASE[g]
        for c in classes:
            W, b = subnets[c]["fc1"]
            W = _f32(W)                     # [108, 2S]
            np.add.at(W1exp[:, off:off + T2[c]], sg[c], W[:100])
            wco[:, off:off + T2[c]] = W[100:108]
            b1[off:off + T2[c]] = _f32(b)
            off += T2[c]

    # hidden layers: block-diagonal per group
    H1 = [_block_diag([_f32(subnets[c]["hidden"][0][0]) for c in g]) for g in GROUPS]
    H2 = [_block_diag([_f32(subnets[c]["hidden"][1][0]) for c in g]) for g in GROUPS]
    bh1 = [np.concatenate([_f32(subnets[c]["hidden"][0][1]) for c in g]) for g in GROUPS]
    bh2 = [np.concatenate([_f32(subnets[c]["hidden"][1][1]) for c in g]) for g in GROUPS]

    # fc2: scatter group rows -> global output columns (0:128 in M0, 128:130 in M1)
    W2M0 = [np.zeros((GSIZE[g], 128), np.float32) for g in range(3)]
    W2M1 = np.zeros((GSIZE[2], 2), np.float32)
    b2 = np.zeros(130, np.float32)
    for g, classes in enumerate(GROUPS):
        lr = 0
        for c in classes:
            W, b = subnets[c]["fc2"]
            W = _f32(W)                     # [2S, S]
            for j in range(SUB_S[c]):
                col = OUT_OFF[c] + j
                if col < 128:
                    W2M0[g][lr:lr + T2[c], col] = W[:, j]
                else:
                    W2M1[lr:lr + T2[c], col - 128] = W[:, j]
            b2[OUT_OFF[c]:OUT_OFF[c] + SUB_S[c]] = _f32(b)
            lr += T2[c]

    # softmax selectors
    cls_of_row = np.zeros(130, np.int64)
    for c in range(NUM_CLASSES):
        cls_of_row[OUT_OFF[c]:OUT_OFF[c + 1]] = c
    SB = np.zeros((130, 8), np.float32)
    SB[np.arange(130), cls_of_row] = 1.0
    BL = SB.T.copy()                        # [8, 130]

    return {
        "W1exp": W1exp, "wco": wco, "b1": b1, "bc1": cb1, "wc2": cW2, "bc2": cb2,
        "H1": H1, "H2": H2, "bh1": bh1, "bh2": bh2,
        "W2M0": W2M0, "W2M1": W2M1, "b2": b2,
        "sb0": SB[:128], "sb1": SB[128:], "bl0": BL[:, :128], "bl1": BL[:, 128:],
        "ones8": np.ones((8, 1), np.float32), "ones1x8": np.ones((1, 8), np.float32),
    }


def numpy_forward(w, x):
    """Reference implementation of the device math (fp32 numpy), for validation."""
    relu = lambda v: np.maximum(v, 0.0)
    zc = x @ w["W1exp"][:, 260:276] + w["bc1"]
    ch = relu(zc)
    z2 = ch @ w["wc2"] + w["bc2"]
    ls = np.log(np.exp(z2).sum(1, keepdims=True))
    co = z2 - ls                                    # class log-probs [n, 8]
    h1 = relu(x @ w["W1exp"][:, :260] + co @ w["wco"] + w["b1"])
    H1 = _block_diag(w["H1"]); H2 = _block_diag(w["H2"])
    h2 = relu(h1 @ H1 + np.concatenate(w["bh1"]))
    h3 = relu(h2 @ H2 + np.concatenate(w["bh2"]))
    h4 = h3 + h1
    z = np.zeros((x.shape[0], 130), np.float32)
    for g in range(3):
        hg = h4[:, GBASE[g]:GBASE[g] + GSIZE[g]]
        z[:, :128] += hg @ w["W2M0"][g]
    z[:, 128:] += h4[:, GBASE[2]:GBASE[2] + GSIZE[2]] @ w["W2M1"]
    z += w["b2"]
    es = np.exp(z)
    SB = np.concatenate([w["sb0"], w["sb1"]], 0)    # [130, 8]
    s = es @ SB                                     # per-class sums [n, 8]
    lss = np.log(s)
    LS = lss @ np.concatenate([w["bl0"], w["bl1"]], 1)   # [n, 130]
    CO = co @ np.concatenate([w["bl0"], w["bl1"]], 1)    # [n, 130]
    return (z - LS) * CO


# ---------------------------------------------------------------------------
# device program
# ---------------------------------------------------------------------------

def _dev_weight_arrays(w):
    """Cast to device dtypes. Matmul operands -> fp16, biases -> fp32 [P,1]."""
    bf = lambda a: np.ascontiguousarray(np.asarray(a, np.float32)).astype(F16)
    col = lambda a: np.ascontiguousarray(np.asarray(a, np.float32).reshape(-1, 1))
    # stacked fc1 K-chunk 3: x rows 240:300 at 0:60, zero pad 60:64, class_out 64:72.
    w1_k2 = np.zeros((72, 276), np.float32)
    w1_k2[0:60] = w["W1exp"][240:300]
    w1_k2[64:72, 0:260] = w["wco"]
    # CO broadcast selectors padded to base-partition 64 (lhsT base must match rhs)
    blp0 = np.zeros((72, 128), np.float32); blp0[64:72] = w["bl0"]
    blp1 = np.zeros((72, 2), np.float32); blp1[64:72] = w["bl1"]
    d = {
        "w1_k0": bf(w["W1exp"][0:120]),
        "w1_k1": bf(w["W1exp"][120:240]),
        "w1_k2": bf(w1_k2),
        "blp0": bf(blp0), "blp1": bf(blp1),
        "wc2": bf(w["wc2"]),
        "h1g0": bf(w["H1"][0]), "h1g1": bf(w["H1"][1]), "h1g2": bf(w["H1"][2]),
        "h2g0": bf(w["H2"][0]), "h2g1": bf(w["H2"][1]), "h2g2": bf(w["H2"][2]),
        "w2m0g0": bf(w["W2M0"][0]), "w2m0g1": bf(w["W2M0"][1]), "w2m0g2": bf(w["W2M0"][2]),
        "w2m1": bf(w["W2M1"]),
        "sb0": bf(w["sb0"]), "sb1": bf(w["sb1"]),
        "bl0": bf(w["bl0"]), "bl1": bf(w["bl1"]),
        "ones8": bf(w["ones8"]), "ones1x8": bf(w["ones1x8"]),
        "bc1_b": col(w["bc1"]), "bc2_b": col(w["bc2"]),
        "b1g0_b": col(w["b1"][0:80]), "b1g1_b": col(w["b1"][80:200]), "b1g2_b": col(w["b1"][200:260]),
        "bh1g0_b": col(w["bh1"][0]), "bh1g1_b": col(w["bh1"][1]), "bh1g2_b": col(w["bh1"][2]),
        "bh2g0_b": col(w["bh2"][0]), "bh2g1_b": col(w["bh2"][1]), "bh2g2_b": col(w["bh2"][2]),
        "b2m0_b": col(w["b2"][0:128]), "b2m1_b": col(w["b2"][128:130]),
    }
    return d


def build_nc(n_tiles=NT, bt=BT):
    import concourse.bacc as bacc
    import concourse.tile as tile
    import concourse.mybir as mybir

    dt = mybir.dt
    AF = mybir.ActivationFunctionType
    ALU = mybir.AluOpType
    B = n_tiles * bt

    nc = bacc.Bacc("TRN2", target_bir_lowering=False, debug=False,
                   num_devices=N_CORES)

    xT = nc.dram_tensor("xT", [304, B], dt.float16, kind="ExternalInput")
    outT = nc.dram_tensor("outT", [130, B], dt.float32, kind="ExternalOutput")

    wspec = {
        "w1_k0": ([120, 276], dt.float16), "w1_k1": ([120, 276], dt.float16),
        "w1_k2": ([72, 276], dt.float16),
        "blp0": ([72, 128], dt.float16), "blp1": ([72, 2], dt.float16),
        "wc2": ([16, 8], dt.float16),
        "h1g0": ([80, 80], dt.float16), "h1g1": ([120, 120], dt.float16), "h1g2": ([60, 60], dt.float16),
        "h2g0": ([80, 80], dt.float16), "h2g1": ([120, 120], dt.float16), "h2g2": ([60, 60], dt.float16),
        "w2m0g0": ([80, 128], dt.float16), "w2m0g1": ([120, 128], dt.float16), "w2m0g2": ([60, 128], dt.float16),
        "w2m1": ([60, 2], dt.float16),
        "sb0": ([128, 8], dt.float16), "sb1": ([2, 8], dt.float16),
        "bl0": ([8, 128], dt.float16), "bl1": ([8, 2], dt.float16),
        "ones8": ([8, 1], dt.float16), "ones1x8": ([1, 8], dt.float16),
        "bc1_b": ([16, 1], dt.float32), "bc2_b": ([8, 1], dt.float32),
        "b1g0_b": ([80, 1], dt.float32), "b1g1_b": ([120, 1], dt.float32), "b1g2_b": ([60, 1], dt.float32),
        "bh1g0_b": ([80, 1], dt.float32), "bh1g1_b": ([120, 1], dt.float32), "bh1g2_b": ([60, 1], dt.float32),
        "bh2g0_b": ([80, 1], dt.float32), "bh2g1_b": ([120, 1], dt.float32), "bh2g2_b": ([60, 1], dt.float32),
        "b2m0_b": ([128, 1], dt.float32), "b2m1_b": ([2, 1], dt.float32),
    }
    wdram = {k: nc.dram_tensor(k, shape, d, kind="ExternalInput")
             for k, (shape, d) in wspec.items()}

    GG = GSIZE                      # [80, 120, 60]
    mm = nc.tensor.matmul

    with tile.TileContext(nc) as tc:
        with tc.tile_pool(name="wp", bufs=1) as wp, \
             tc.tile_pool(name="io", bufs=3) as io, \
             tc.tile_pool(name="iox2", bufs=4) as iox2, \
             tc.tile_pool(name="act", bufs=2) as acp, \
             tc.tile_pool(name="outp", bufs=3) as outp, \
             tc.tile_pool(name="ps", bufs=6, space="PSUM") as ps, \
             tc.tile_pool(name="pst", bufs=1, space="PSUM") as pst:

            # pin the activation table set that holds Relu+Exp+Ln+Identity+Copy
            nc.scalar.add_instruction(mybir.InstLoadActFuncSet(
                name=nc.get_next_instruction_name(), act_func_set_id=6,
                ins=[], outs=[]))

            wt = {}
            for k, (shape, d) in wspec.items():
                t = wp.tile(shape, d, tag=k, name=k)
                nc.sync.dma_start(out=t[:], in_=wdram[k].ap())
                wt[k] = t

            state = {}

            def p0(it):
                c0, c1 = it * bt, (it + 1) * bt
                st = {}
                x0 = io.tile([120, bt], dt.float16, tag="x0", name="x0")
                x1 = io.tile([120, bt], dt.float16, tag="x1", name="x1")
                x2 = iox2.tile([72, bt], dt.float16, tag="x2", name="x2")
                nc.sync.dma_start(out=x0[:], in_=xT.ap()[0:120, c0:c1])
                nc.sync.dma_start(out=x1[:], in_=xT.ap()[120:240, c0:c1])
                nc.sync.dma_start(out=x2[0:64, :], in_=xT.ap()[240:304, c0:c1])

                # fc1 G0 + class head packed into one PSUM bank
                ps0 = ps.tile([112, bt], dt.float32, tag="big", name="ps0")
                mm(ps0[0:80, :], wt["w1_k0"][:, 0:80], x0[:], start=True, stop=False)
                mm(ps0[96:112, :], wt["w1_k0"][:, 260:276], x0[:], start=True,
                   stop=False, tile_position=(0, 96))
                mm(ps0[0:80, :], wt["w1_k1"][:, 0:80], x1[:], start=False, stop=False)
                mm(ps0[96:112, :], wt["w1_k1"][:, 260:276], x1[:], start=False,
                   stop=False, tile_position=(0, 96))
                mm(ps0[96:112, :], wt["w1_k2"][0:60, 260:276], x2[0:60, :],
                   start=False, stop=True, tile_position=(0, 96))
                psG1 = ps.tile([120, bt], dt.float32, tag="big", name="psG1")
                psG2 = ps.tile([60, bt], dt.float32, tag="big", name="psG2")
                mm(psG1[:], wt["w1_k0"][:, 80:200], x0[:], start=True, stop=False)
                mm(psG1[:], wt["w1_k1"][:, 80:200], x1[:], start=False, stop=False)
                mm(psG2[:], wt["w1_k0"][:, 200:260], x0[:], start=True, stop=False)
                mm(psG2[:], wt["w1_k1"][:, 200:260], x1[:], start=False, stop=False)

                # class softmax chain
                ch = acp.tile([16, bt], dt.float16, tag="ch", name="ch")
                nc.scalar.activation(ch[:], ps0[96:112, :], AF.Relu, bias=wt["bc1_b"][:])
                tailA = pst.tile([128, bt], dt.float32, tag="tailA", name="tailA")
                psB = tailA[0:8, :]
                psC = tailA[32:33, :]
                psD = tailA[64:72, :]
                mm(psB, wt["wc2"][:], ch[:], start=True, stop=True)
                e_cls = acp.tile([8, bt], dt.float16, tag="e_cls", name="e_cls")
                nc.scalar.activation(e_cls[:], psB, AF.Exp, bias=wt["bc2_b"][:])
                z_cls = acp.tile([8, bt], dt.float32, tag="z_cls", name="z_cls")
                nc.vector.tensor_scalar_add(z_cls[:], psB, wt["bc2_b"][:])
                mm(psC, wt["ones8"][:], e_cls[:], start=True, stop=True,
                   tile_position=(0, 32))
                ls_cls = acp.tile([1, bt], dt.float16, tag="ls_cls", name="ls_cls")
                nc.scalar.activation(ls_cls[:], psC, AF.Ln)
                mm(psD, wt["ones1x8"][:], ls_cls[:], start=True, stop=True,
                   tile_position=(0, 64))
                nc.vector.tensor_sub(x2[64:72, :], z_cls[:], psD)

                # finish fc1 with the stacked chunk (x tail + class_out)
                mm(ps0[0:80, :], wt["w1_k2"][:, 0:80], x2[:], start=False, stop=True)
                mm(psG1[:], wt["w1_k2"][:, 80:200], x2[:], start=False, stop=True)
                mm(psG2[:], wt["w1_k2"][:, 200:260], x2[:], start=False, stop=True)

                h1 = []
                for g, pg in enumerate([ps0[0:80, :], psG1[:], psG2[:]]):
                    t = acp.tile([GG[g], bt], dt.float16, tag="h1g%d" % g,
                                 name="h1g%d" % g)
                    nc.scalar.activation(t[:], pg, AF.Relu, bias=wt["b1g%d_b" % g][:])
                    h1.append(t)
                st["x2"] = x2
                st["h1"] = h1
                state[it] = st

            def p1(it):
                st = state[it]
                h1 = st["h1"]
                # hidden 1
                h2 = []
                for g in range(3):
                    pg = ps.tile([GG[g], bt], dt.float32, tag="big", name="psH%d" % g)
                    mm(pg[:], wt["h1g%d" % g][:], h1[g][:], start=True, stop=True)
                    t = acp.tile([GG[g], bt], dt.float16, tag="h2g%d" % g,
                                 name="h2g%d" % g)
                    if g == 1:
                        nc.scalar.activation(t[:], pg[:], AF.Relu,
                                             bias=wt["bh1g%d_b" % g][:])
                    else:
                        nc.vector.tensor_scalar(t[:], pg[:], wt["bh1g%d_b" % g][:],
                                                0.0, ALU.add, ALU.max)
                    h2.append(t)
                # hidden 2 + skip
                h4 = []
                for g in range(3):
                    pg = ps.tile([GG[g], bt], dt.float32, tag="big", name="psI%d" % g)
                    mm(pg[:], wt["h2g%d" % g][:], h2[g][:], start=True, stop=True)
                    t3 = acp.tile([GG[g], bt], dt.float16, tag="h3g%d" % g,
                                  name="h3g%d" % g)
                    nc.vector.tensor_scalar(t3[:], pg[:], wt["bh2g%d_b" % g][:],
                                            0.0, ALU.add, ALU.max)
                    t4 = acp.tile([GG[g], bt], dt.float16, tag="h4g%d" % g,
                                  name="h4g%d" % g)
                    nc.gpsimd.tensor_add(t4[:], t3[:], h1[g][:])
                    h4.append(t4)
                # fc2
                psZ0 = ps.tile([128, bt], dt.float32, tag="big", name="psZ0")
                for g in range(3):
                    mm(psZ0[:], wt["w2m0g%d" % g][:], h4[g][:],
                       start=(g == 0), stop=(g == 2))
                tailB = pst.tile([128, bt], dt.float32, tag="tailB", name="tailB")
                mm(tailB[0:2, :], wt["w2m1"][:], h4[2][:], start=True, stop=True)
                e0 = acp.tile([128, bt], dt.float16, tag="e0", name="e0")
                e1 = acp.tile([2, bt], dt.float16, tag="e1", name="e1")
                nc.scalar.activation(e0[:], psZ0[:], AF.Exp, bias=wt["b2m0_b"][:])
                nc.scalar.activation(e1[:], tailB[0:2, :], AF.Exp, bias=wt["b2m1_b"][:])
                z0 = acp.tile([128, bt], dt.float32, tag="z0", name="z0")
                z1 = acp.tile([2, bt], dt.float32, tag="z1", name="z1")
                nc.vector.tensor_scalar_add(z0[:], psZ0[:], wt["b2m0_b"][:])
                nc.vector.tensor_scalar_add(z1[:], tailB[0:2, :], wt["b2m1_b"][:])
                st["tailB"] = tailB
                st["e"] = (e0, e1)
                st["z"] = (z0, z1)

            def p2(it):
                c0, c1 = it * bt, (it + 1) * bt
                st = state.pop(it)
                tailB = st["tailB"]
                e0, e1 = st["e"]
                z0, z1 = st["z"]
                co = st["x2"][64:72, :]
                psS = tailB[32:40, :]
                mm(psS, wt["sb0"][:], e0[:], start=True, stop=False,
                   tile_position=(0, 32))
                mm(psS, wt["sb1"][:], e1[:], start=False, stop=True,
                   tile_position=(0, 32))
                ls_sub = acp.tile([8, bt], dt.float16, tag="ls_sub", name="ls_sub")
                nc.scalar.activation(ls_sub[:], psS, AF.Ln)
                psL0 = ps.tile([128, bt], dt.float32, tag="big", name="psL0")
                mm(psL0[:], wt["bl0"][:], ls_sub[:], start=True, stop=True)
                mm(tailB[64:66, :], wt["bl1"][:], ls_sub[:], start=True, stop=True,
                   tile_position=(0, 64))
                psO0 = ps.tile([128, bt], dt.float32, tag="big", name="psO0")
                mm(psO0[:], wt["blp0"][64:72, :], co, start=True, stop=True)
                mm(tailB[96:98, :], wt["blp1"][64:72, :], co, start=True, stop=True,
                   tile_position=(64, 96))
                d0 = acp.tile([128, bt], dt.float32, tag="d0", name="d0")
                d1 = acp.tile([2, bt], dt.float32, tag="d1", name="d1")
                nc.vector.tensor_sub(d0[:], z0[:], psL0[:])
                nc.vector.tensor_sub(d1[:], z1[:], tailB[64:66, :])
                o0 = outp.tile([128, bt], dt.float32, tag="o0", name="o0")
                o1 = outp.tile([2, bt], dt.float32, tag="o1", name="o1")
                nc.vector.tensor_mul(o0[:], d0[:], psO0[:])
                nc.vector.tensor_mul(o1[:], d1[:], tailB[96:98, :])
                nc.sync.dma_start(out=outT.ap()[0:128, c0:c1], in_=o0[:])
                nc.sync.dma_start(out=outT.ap()[128:130, c0:c1], in_=o1[:])

            for step in range(n_tiles + 2):
                if step < n_tiles:
                    p0(step)
                if 1 <= step <= n_tiles:
                    p1(step - 1)
                if step >= 2:
                    p2(step - 2)

    nc.finalize()
    return nc


_EXEC_TIME_NS = None


def kernel(x, class_gene_idx, sub_gene_idx, params):
    global _EXEC_TIME_NS
    import os
    from concourse import bass_utils

    x = np.asarray(x)
    w = prep_weights(class_gene_idx, sub_gene_idx, params)
    dev_w = _dev_weight_arrays(w)

    xbf = np.asarray(x, np.float32).astype(F16)        # [262144, 300] fp16

    nc = build_nc(NT, BT)

    in_maps = []
    for c in range(N_CORES):
        sl = xbf[c * B_CORE:(c + 1) * B_CORE].T            # [300, 32768]
        slab = np.zeros((304, B_CORE), F16)
        slab[0:300] = sl                                    # rows 300:304 stay zero
        in_maps.append({**dev_w, "xT": slab})

    trace = bool(int(os.environ.get("CUTTLE_TRACE", "0")))
    res = bass_utils.run_bass_kernel_spmd(nc, in_maps, core_ids=list(range(N_CORES)),
                                          trace=trace)
    _EXEC_TIME_NS = res.exec_time_ns

    out = np.empty((B_TOTAL, 130), np.float32)
    for c in range(N_CORES):
        out[c * B_CORE:(c + 1) * B_CORE] = res.results[c]["outT"].T
    return out


# revision 12
# speedup vs baseline: 1.7946x; 1.1397x over previous
"""CuttleNet Trainium2 kernel.

Strategy:
- Data-parallel over the 262144-row batch across 8 NeuronCores (32768 rows each).
- Host folds the gene-column gathers into expanded weight matrices (scatter-add
  over the index tables), so the device only does dense matmuls on x.
- Everything on device is computed feature-major ([features, batch] with batch on
  the free dimension, 512-column tiles = one PSUM bank). The host pre-transposes
  x to xT [300, B] (bf16) and post-transposes the [130, B] fp32 result back.
- The 8 per-class MLPs are packed into 3 partition groups ({c0}=80, {c1..c4}=120,
  {c5,c6,c7}=60 rows of 2S), hidden layers become block-diagonal matmuls, and
  log-softmax is done with ones-matmul partition reductions plus selector-matmul
  broadcasts of the per-class log-sums / class log-probs.
"""

import numpy as np
import ml_dtypes

F16 = np.float16

NUM_CLASSES = 8
SUB_S = [40, 30, 15, 10, 5, 10, 19, 1]          # S_c, sums to 130
T2 = [2 * s for s in SUB_S]                      # 2*S_c, sums to 260
GROUPS = [[0], [1, 2, 3, 4], [5, 6, 7]]
GSIZE = [sum(T2[c] for c in g) for g in GROUPS]  # [80, 120, 60]
GBASE = [0, 80, 200]                             # col offset of group in the 260
OUT_OFF = np.concatenate([[0], np.cumsum(SUB_S)])  # [0,40,70,85,95,100,110,129,130]
N_CORES = 8
B_TOTAL = 262144
B_CORE = B_TOTAL // N_CORES                      # 32768
BT = 512                                         # batch tile (one PSUM bank fp32)
NT = B_CORE // BT                                # 64 tiles per core


def _f32(a):
    return np.asarray(a, dtype=np.float32)


def _block_diag(mats):
    n = sum(m.shape[0] for m in mats)
    k = sum(m.shape[1] for m in mats)
    out = np.zeros((n, k), np.float32)
    r = c = 0
    for m in mats:
        out[r:r + m.shape[0], c:c + m.shape[1]] = m
        r += m.shape[0]
        c += m.shape[1]
    return out


def prep_weights(class_gene_idx, sub_gene_idx, params):
    """Expand/repack all model weights into the device layout (fp32)."""
    cg = np.asarray(class_gene_idx)
    sg = np.asarray(sub_gene_idx)
    cW1, cb1 = (_f32(a) for a in params["class_fc1"])   # [60,16], [16]
    cW2, cb2 = (_f32(a) for a in params["class_fc2"])   # [16,8], [8]
    subnets = params["subnets"]

    # fc1, expanded over gene gathers. cols 0:260 subnets (group order), 260:276 class head
    W1exp = np.zeros((300, 276), np.float32)
    np.add.at(W1exp[:, 260:276], cg, cW1)
    wco = np.zeros((8, 260), np.float32)   # class_out -> subnet fc1 contribution
    b1 = np.zeros(260, np.float32)
    for g, classes in enumerate(GROUPS):
        off = GBASE[g]
        for c in classes:
            W, b = subnets[c]["fc1"]
            W = _f32(W)                     # [108, 2S]
            np.add.at(W1exp[:, off:off + T2[c]], sg[c], W[:100])
            wco[:, off:off + T2[c]] = W[100:108]
            b1[off:off + T2[c]] = _f32(b)
            off += T2[c]

    # hidden layers: block-diagonal per group
    H1 = [_block_diag([_f32(subnets[c]["hidden"][0][0]) for c in g]) for g in GROUPS]
    H2 = [_block_diag([_f32(subnets[c]["hidden"][1][0]) for c in g]) for g in GROUPS]
    bh1 = [np.concatenate([_f32(subnets[c]["hidden"][0][1]) for c in g]) for g in GROUPS]
    bh2 = [np.concatenate([_f32(subnets[c]["hidden"][1][1]) for c in g]) for g in GROUPS]

    # fc2: scatter group rows -> global output columns (0:128 in M0, 128:130 in M1)
    W2M0 = [np.zeros((GSIZE[g], 128), np.float32) for g in range(3)]
    W2M1 = np.zeros((GSIZE[2], 2), np.float32)
    b2 = np.zeros(130, np.float32)
    for g, classes in enumerate(GROUPS):
        lr = 0
        for c in classes:
            W, b = subnets[c]["fc2"]
            W = _f32(W)                     # [2S, S]
            for j in range(SUB_S[c]):
                col = OUT_OFF[c] + j
                if col < 128:
                    W2M0[g][lr:lr + T2[c], col] = W[:, j]
                else:
                    W2M1[lr:lr + T2[c], col - 128] = W[:, j]
            b2[OUT_OFF[c]:OUT_OFF[c] + SUB_S[c]] = _f32(b)
            lr += T2[c]

    # softmax selectors
    cls_of_row = np.zeros(130, np.int64)
    for c in range(NUM_CLASSES):
        cls_of_row[OUT_OFF[c]:OUT_OFF[c + 1]] = c
    SB = np.zeros((130, 8), np.float32)
    SB[np.arange(130), cls_of_row] = 1.0
    BL = SB.T.copy()                        # [8, 130]

    return {
        "W1exp": W1exp, "wco": wco, "b1": b1, "bc1": cb1, "wc2": cW2, "bc2": cb2,
        "H1": H1, "H2": H2, "bh1": bh1, "bh2": bh2,
        "W2M0": W2M0, "W2M1": W2M1, "b2": b2,
        "sb0": SB[:128], "sb1": SB[128:], "bl0": BL[:, :128], "bl1": BL[:, 128:],
        "ones8": np.ones((8, 1), np.float32), "ones1x8": np.ones((1, 8), np.float32),
    }


def numpy_forward(w, x):
    """Reference implementation of the device math (fp32 numpy), for validation."""
    relu = lambda v: np.maximum(v, 0.0)
    zc = x @ w["W1exp"][:, 260:276] + w["bc1"]
    ch = relu(zc)
    z2 = ch @ w["wc2"] + w["bc2"]
    ls = np.log(np.exp(z2).sum(1, keepdims=True))
    co = z2 - ls                                    # class log-probs [n, 8]
    h1 = relu(x @ w["W1exp"][:, :260] + co @ w["wco"] + w["b1"])
    H1 = _block_diag(w["H1"]); H2 = _block_diag(w["H2"])
    h2 = relu(h1 @ H1 + np.concatenate(w["bh1"]))
    h3 = relu(h2 @ H2 + np.concatenate(w["bh2"]))
    h4 = h3 + h1
    z = np.zeros((x.shape[0], 130), np.float32)
    for g in range(3):
        hg = h4[:, GBASE[g]:GBASE[g] + GSIZE[g]]
        z[:, :128] += hg @ w["W2M0"][g]
    z[:, 128:] += h4[:, GBASE[2]:GBASE[2] + GSIZE[2]] @ w["W2M1"]
    z += w["b2"]
    es = np.exp(z)
    SB = np.concatenate([w["sb0"], w["sb1"]], 0)    # [130, 8]
    s = es @ SB                                     # per-class sums [n, 8]
    lss = np.log(s)
    LS = lss @ np.concatenate([w["bl0"], w["bl1"]], 1)   # [n, 130]
    CO = co @ np.concatenate([w["bl0"], w["bl1"]], 1)    # [n, 130]
    return (z - LS) * CO


# ---------------------------------------------------------------------------
# device program
# ---------------------------------------------------------------------------

def _dev_weight_arrays(w):
    """Cast to device dtypes. Matmul operands -> fp16, biases -> fp32 [P,1]."""
    bf = lambda a: np.ascontiguousarray(np.asarray(a, np.float32)).astype(F16)
    col = lambda a: np.ascontiguousarray(np.asarray(a, np.float32).reshape(-1, 1))
    # stacked fc1 K-chunk 3: x rows 240:300 at 0:60, zero pad 60:64, class_out 64:72.
    w1_k2 = np.zeros((72, 276), np.float32)
    w1_k2[0:60] = w["W1exp"][240:300]
    w1_k2[64:72, 0:260] = w["wco"]
    # CO broadcast selectors padded to base-partition 64 (lhsT base must match rhs)
    blp0 = np.zeros((72, 128), np.float32); blp0[64:72] = w["bl0"]
    blp1 = np.zeros((72, 2), np.float32); blp1[64:72] = w["bl1"]
    # negative log-softmax accumulators: psum += lhsT.T @ [ones; ls_rows]
    # row 0 pairs with the persistent ones row (+bias), rows 1.. with the ln rows (-select)
    nblc = np.zeros((2, 8), np.float32)
    nblc[0] = -1.0; nblc[1] = w["bc2"]
    nbl0 = np.zeros((9, 128), np.float32)
    nbl0[0:8] = -w["bl0"]; nbl0[8] = w["b2"][0:128]
    nbl1 = np.zeros((9, 2), np.float32)
    nbl1[0:8] = -w["bl1"]; nbl1[8] = w["b2"][128:130]
    d = {
        "w1_k0": bf(w["W1exp"][0:120]),
        "w1_k1": bf(w["W1exp"][120:240]),
        "w1_k2": bf(w1_k2),
        "blp0": bf(blp0), "blp1": bf(blp1),
        "nblc": bf(nblc), "nbl0": bf(nbl0), "nbl1": bf(nbl1),
        "wc2": bf(w["wc2"]),
        "h1g0": bf(w["H1"][0]), "h1g1": bf(w["H1"][1]), "h1g2": bf(w["H1"][2]),
        "h2g0": bf(w["H2"][0]), "h2g1": bf(w["H2"][1]), "h2g2": bf(w["H2"][2]),
        "w2m0g0": bf(w["W2M0"][0]), "w2m0g1": bf(w["W2M0"][1]), "w2m0g2": bf(w["W2M0"][2]),
        "w2m1": bf(w["W2M1"]),
        "sb0": bf(w["sb0"]), "sb1": bf(w["sb1"]),
        "ones8": bf(w["ones8"]),
        "bc1_b": col(w["bc1"]), "bc2_b": col(w["bc2"]),
        "b1g0_b": col(w["b1"][0:80]), "b1g1_b": col(w["b1"][80:200]), "b1g2_b": col(w["b1"][200:260]),
        "bh1g0_b": col(w["bh1"][0]), "bh1g1_b": col(w["bh1"][1]), "bh1g2_b": col(w["bh1"][2]),
        "bh2g0_b": col(w["bh2"][0]), "bh2g1_b": col(w["bh2"][1]), "bh2g2_b": col(w["bh2"][2]),
        "b2m0_b": col(w["b2"][0:128]), "b2m1_b": col(w["b2"][128:130]),
    }
    return d


def build_nc(n_tiles=NT, bt=BT):
    import concourse.bacc as bacc
    import concourse.tile as tile
    import concourse.mybir as mybir

    dt = mybir.dt
    AF = mybir.ActivationFunctionType
    ALU = mybir.AluOpType
    B = n_tiles * bt

    nc = bacc.Bacc("TRN2", target_bir_lowering=False, debug=False,
                   num_devices=N_CORES)

    xT = nc.dram_tensor("xT", [304, B], dt.float16, kind="ExternalInput")
    outT = nc.dram_tensor("outT", [130, B], dt.float32, kind="ExternalOutput")

    wspec = {
        "w1_k0": ([120, 276], dt.float16), "w1_k1": ([120, 276], dt.float16),
        "w1_k2": ([72, 276], dt.float16),
        "blp0": ([72, 128], dt.float16), "blp1": ([72, 2], dt.float16),
        "nblc": ([2, 8], dt.float16), "nbl0": ([9, 128], dt.float16),
        "nbl1": ([9, 2], dt.float16),
        "wc2": ([16, 8], dt.float16),
        "h1g0": ([80, 80], dt.float16), "h1g1": ([120, 120], dt.float16), "h1g2": ([60, 60], dt.float16),
        "h2g0": ([80, 80], dt.float16), "h2g1": ([120, 120], dt.float16), "h2g2": ([60, 60], dt.float16),
        "w2m0g0": ([80, 128], dt.float16), "w2m0g1": ([120, 128], dt.float16), "w2m0g2": ([60, 128], dt.float16),
        "w2m1": ([60, 2], dt.float16),
        "sb0": ([128, 8], dt.float16), "sb1": ([2, 8], dt.float16),
        "ones8": ([8, 1], dt.float16),
        "bc1_b": ([16, 1], dt.float32), "bc2_b": ([8, 1], dt.float32),
        "b1g0_b": ([80, 1], dt.float32), "b1g1_b": ([120, 1], dt.float32), "b1g2_b": ([60, 1], dt.float32),
        "bh1g0_b": ([80, 1], dt.float32), "bh1g1_b": ([120, 1], dt.float32), "bh1g2_b": ([60, 1], dt.float32),
        "bh2g0_b": ([80, 1], dt.float32), "bh2g1_b": ([120, 1], dt.float32), "bh2g2_b": ([60, 1], dt.float32),
        "b2m0_b": ([128, 1], dt.float32), "b2m1_b": ([2, 1], dt.float32),
    }
    wdram = {k: nc.dram_tensor(k, shape, d, kind="ExternalInput")
             for k, (shape, d) in wspec.items()}

    GG = GSIZE                      # [80, 120, 60]
    mm = nc.tensor.matmul

    with tile.TileContext(nc) as tc:
        with tc.tile_pool(name="wp", bufs=1) as wp, \
             tc.tile_pool(name="io", bufs=3) as io, \
             tc.tile_pool(name="iox2", bufs=7) as iox2, \
             tc.tile_pool(name="acp", bufs=3) as acp, \
             tc.tile_pool(name="h1p", bufs=4) as h1p, \
             tc.tile_pool(name="outp", bufs=3) as outp, \
             tc.tile_pool(name="ps", bufs=6, space="PSUM") as ps, \
             tc.tile_pool(name="pst", bufs=1, space="PSUM") as pstA, \
             tc.tile_pool(name="pstB", bufs=1, space="PSUM") as pstB:

            # pin the activation table set that holds Relu+Exp+Ln+Identity+Copy
            nc.scalar.add_instruction(mybir.InstLoadActFuncSet(
                name=nc.get_next_instruction_name(), act_func_set_id=6,
                ins=[], outs=[]))

            wt = {}
            for k, (shape, d) in wspec.items():
                t = wp.tile(shape, d, tag=k, name=k)
                nc.sync.dma_start(out=t[:], in_=wdram[k].ap())
                wt[k] = t

            # persistent log-sum tiles: row 0 = ones (for bias folding via the
            # negative-accumulate matmuls), remaining rows rewritten per tile.
            ls_clsp = wp.tile([2, bt], dt.float16, tag="ls_clsp", name="ls_clsp")
            ls_subp = wp.tile([9, bt], dt.float16, tag="ls_subp", name="ls_subp")
            nc.vector.memset(ls_clsp[:], 1.0)
            nc.vector.memset(ls_subp[:], 1.0)

            state = {}

            def p0a(it):
                c0, c1 = it * bt, (it + 1) * bt
                st = {}
                x0 = io.tile([120, bt], dt.float16, tag="x0", name="x0")
                x1 = io.tile([120, bt], dt.float16, tag="x1", name="x1")
                x2 = iox2.tile([72, bt], dt.float16, tag="x2", name="x2")
                nc.sync.dma_start(out=x0[:], in_=xT.ap()[0:120, c0:c1])
                nc.sync.dma_start(out=x1[:], in_=xT.ap()[120:240, c0:c1])
                nc.sync.dma_start(out=x2[0:64, :], in_=xT.ap()[240:304, c0:c1])

                # class head: fc1 into tailA[0:16], softmax folded into psB
                tailA = pstA.tile([128, bt], dt.float32, tag="tailA", name="tailA")
                psA = tailA[0:16, :]
                psB = tailA[32:40, :]
                psC = tailA[64:65, :]
                mm(psA, wt["w1_k0"][:, 260:276], x0[:], start=True, stop=False)
                mm(psA, wt["w1_k1"][:, 260:276], x1[:], start=False, stop=False)
                mm(psA, wt["w1_k2"][0:60, 260:276], x2[0:60, :], start=False, stop=True)
                ch = acp.tile([16, bt], dt.float16, tag="ch", name="ch")
                nc.scalar.activation(ch[:], psA, AF.Relu, bias=wt["bc1_b"][:])
                mm(psB, wt["wc2"][:], ch[:], start=True, stop=False)
                e_cls = acp.tile([8, bt], dt.float16, tag="e_cls", name="e_cls")
                nc.scalar.activation(e_cls[:], psB, AF.Exp, bias=wt["bc2_b"][:])
                mm(psC, wt["ones8"][:], e_cls[:], start=True, stop=True)
                nc.scalar.activation(ls_clsp[0:1, :], psC, AF.Ln)
                mm(psB, wt["nblc"][:], ls_clsp[:], start=False, stop=True)
                # class_out (fp16) lands in the stacked x2 rows 64:72
                nc.vector.tensor_copy(x2[64:72, :], psB)
                st["x2"] = x2
                st["x01"] = (x0, x1)
                state[it] = st

            def p0b(it):
                st = state[it]
                x0, x1 = st.pop("x01")
                x2 = st["x2"]
                h1 = []
                for g, (lo, hi) in enumerate([(0, 80), (80, 200), (200, 260)]):
                    pg = ps.tile([GG[g], bt], dt.float32, tag="big", name="psG%d" % g)
                    mm(pg[:], wt["w1_k0"][:, lo:hi], x0[:], start=True, stop=False)
                    mm(pg[:], wt["w1_k1"][:, lo:hi], x1[:], start=False, stop=False)
                    mm(pg[:], wt["w1_k2"][:, lo:hi], x2[:], start=False, stop=True)
                    t = h1p.tile([GG[g], bt], dt.float16, tag="h1g%d" % g,
                                 name="h1g%d" % g)
                    nc.scalar.activation(t[:], pg[:], AF.Relu, bias=wt["b1g%d_b" % g][:])
                    h1.append(t)
                st["h1"] = h1

            def p1a(it):
                st = state[it]
                h1 = st["h1"]
                h2 = []
                for g in range(3):
                    pg = ps.tile([GG[g], bt], dt.float32, tag="big", name="psH%d" % g)
                    mm(pg[:], wt["h1g%d" % g][:], h1[g][:], start=True, stop=True)
                    t = acp.tile([GG[g], bt], dt.float16, tag="h2g%d" % g,
                                 name="h2g%d" % g)
                    nc.vector.tensor_scalar(t[:], pg[:], wt["bh1g%d_b" % g][:],
                                            0.0, ALU.add, ALU.max)
                    h2.append(t)
                st["h2"] = h2

            def p1b(it):
                st = state[it]
                h1 = st.pop("h1")
                h2 = st.pop("h2")
                h4 = []
                for g in range(3):
                    pg = ps.tile([GG[g], bt], dt.float32, tag="big", name="psI%d" % g)
                    mm(pg[:], wt["h2g%d" % g][:], h2[g][:], start=True, stop=True)
                    t3 = acp.tile([GG[g], bt], dt.float16, tag="h3g%d" % g,
                                  name="h3g%d" % g)
                    nc.vector.tensor_scalar(t3[:], pg[:], wt["bh2g%d_b" % g][:],
                                            0.0, ALU.add, ALU.max)
                    t4 = acp.tile([GG[g], bt], dt.float16, tag="h4g%d" % g,
                                  name="h4g%d" % g)
                    nc.gpsimd.tensor_add(t4[:], t3[:], h1[g][:])
                    h4.append(t4)
                st["h4"] = h4

            def p2(it):
                c0, c1 = it * bt, (it + 1) * bt
                st = state.pop(it)
                h4 = st["h4"]
                x2 = st["x2"]
                co = x2[64:72, :]

                psZ0 = ps.tile([128, bt], dt.float32, tag="big", name="psZ0")
                for g in range(3):
                    mm(psZ0[:], wt["w2m0g%d" % g][:], h4[g][:],
                       start=(g == 0), stop=False)
                tailB = pstB.tile([128, bt], dt.float32, tag="tailB", name="tailB")
                psZ1 = tailB[0:2, :]
                psS = tailB[32:40, :]
                psO1 = tailB[64:66, :]
                mm(psZ1, wt["w2m1"][:], h4[2][:], start=True, stop=False)
                e0 = acp.tile([128, bt], dt.float16, tag="e0", name="e0")
                e1 = acp.tile([2, bt], dt.float16, tag="e1", name="e1")
                nc.scalar.activation(e0[:], psZ0[:], AF.Exp, bias=wt["b2m0_b"][:])
                nc.scalar.activation(e1[:], psZ1, AF.Exp, bias=wt["b2m1_b"][:])
                mm(psS, wt["sb0"][:], e0[:], start=True, stop=False)
                mm(psS, wt["sb1"][:], e1[:], start=False, stop=True)
                nc.scalar.activation(ls_subp[0:8, :], psS, AF.Ln)
                # z + b2 - broadcast(ls), accumulated straight into the logits psum
                mm(psZ0[:], wt["nbl0"][:], ls_subp[:], start=False, stop=True)
                mm(psZ1, wt["nbl1"][:], ls_subp[:], start=False, stop=True)
                # class-prob broadcast
                psO0 = ps.tile([128, bt], dt.float32, tag="big", name="psO0")
                mm(psO0[:], wt["blp0"][64:72, :], co, start=True, stop=True)
                mm(psO1, wt["blp1"][64:72, :], co, start=True, stop=True)
                zls0 = acp.tile([128, bt], dt.float32, tag="zls0", name="zls0")
                zls1 = acp.tile([2, bt], dt.float32, tag="zls1", name="zls1")
                nc.scalar.copy(zls0[:], psZ0[:])
                nc.vector.tensor_copy(zls1[:], psZ1)
                o0 = outp.tile([128, bt], dt.float32, tag="o0", name="o0")
                o1 = outp.tile([2, bt], dt.float32, tag="o1", name="o1")
                nc.vector.tensor_mul(o0[:], zls0[:], psO0[:])
                nc.vector.tensor_mul(o1[:], zls1[:], psO1)
                nc.sync.dma_start(out=outT.ap()[0:128, c0:c1], in_=o0[:])
                nc.sync.dma_start(out=outT.ap()[128:130, c0:c1], in_=o1[:])

            stages = [p0a, p0b, p1a, p1b, p2]
            D = len(stages)
            for step in range(n_tiles + D - 1):
                for s, fn in enumerate(stages):
                    it = step - s
                    if 0 <= it < n_tiles:
                        fn(it)

    nc.finalize()
    return nc


_EXEC_TIME_NS = None


def kernel(x, class_gene_idx, sub_gene_idx, params):
    global _EXEC_TIME_NS
    import os
    from concourse import bass_utils

    x = np.asarray(x)
    w = prep_weights(class_gene_idx, sub_gene_idx, params)
    dev_w = _dev_weight_arrays(w)

    xbf = np.asarray(x, np.float32).astype(F16)        # [262144, 300] fp16

    nc = build_nc(NT, BT)

    in_maps = []
    for c in range(N_CORES):
        sl = xbf[c * B_CORE:(c + 1) * B_CORE].T            # [300, 32768]
        slab = np.zeros((304, B_CORE), F16)
        slab[0:300] = sl                                    # rows 300:304 stay zero
        in_maps.append({**dev_w, "xT": slab})

    trace = bool(int(os.environ.get("CUTTLE_TRACE", "0")))
    res = bass_utils.run_bass_kernel_spmd(nc, in_maps, core_ids=list(range(N_CORES)),
                                          trace=trace)
    _EXEC_TIME_NS = res.exec_time_ns

    out = np.empty((B_TOTAL, 130), np.float32)
    for c in range(N_CORES):
        out[c * B_CORE:(c + 1) * B_CORE] = res.results[c]["outT"].T
    return out
